# revision 16
# baseline (speedup 1.0000x reference)
"""NeuromorphicBrainZone Trainium2 kernel (8 NeuronCores, Bass/Tile).

Math (per reference):
    x2 = x.reshape(T, D)                                   # T=1024, D=512
    zone[t, j]  = b_in[j] - mean_d |x2[t, d] - W_in[j, d]|   # N=2048
    spikes      = sigmoid(SURR_BETA * (zone - v_th))
    out[t, m]   = b_out[m] - mean_j |spikes[t, j] - W_out[m, j]|

Sharding: layer-1 neuron dim j is sharded 8 ways (256 j per core, all
tokens). Layer 2 reduces over j, so each core computes partial sums over
its local j for ALL (t, m) and a ReduceScatter(add) over cores both
completes the j-reduction and leaves each core with an m-shard (64 m) of
the output. Host stitches/transposes.

On-core layout: the reduce dim (d for L1, j for L2) lives on SBUF
partitions. Elementwise |in - w| runs on ACT (activation Abs with
per-partition bias = -w column) and DVE (tensor_scalar fused
add + abs_max), in bf16. The partition-reduction runs on the PE as a
matmul with a shifted ones-column lhsT window so row j of PSUM
accumulates sum_d |x - w_j| — all 128 rows of a block accumulate in one
PSUM tile, evacuated by a single ACT op (fused sigmoid / bias+scale).
"""

import sys

sys.path.insert(0, "/opt/trn_rl_repo")

from contextlib import ExitStack

import numpy as np

import concourse.bass as bass
import concourse.bacc as bacc
import concourse.mybir as mybir
import concourse.tile as tile

SURR_BETA = 4.0


def build_kernel(n_cores=8, T=1024, D=512, N=2048, M=512, dve_mod=6, dve_rem=(0,)):
    """Build the per-core Bass program (SPMD: same program, per-core data).

    dve_mod/dve_rem: elementwise units with unit_idx % dve_mod in dve_rem
    go to the DVE; the rest go to ACT. An ACT unit is one fused
    Abs(x - w) activation (one PE stream); a DVE unit is a max(x, w) and
    a min(x, w) tensor_scalar pair whose difference is formed by the PE
    via +1/-1 ones-column windows (two PE streams).
    """
    JC = N // n_cores          # local neurons
    MS = M // n_cores          # output m-shard
    n_dblk = D // 128
    n_jblk = JC // 128
    n_mblk = M // 128
    CH = 512                   # matmul free-dim chunk (one PSUM bank)
    n_ch = (T + CH - 1) // CH
    bf16 = mybir.dt.bfloat16
    f32 = mybir.dt.float32
    Act = mybir.ActivationFunctionType

    nc = bacc.Bacc("TRN2", target_bir_lowering=False, debug=False,
                   num_devices=n_cores)

    xT_d = nc.dram_tensor("xT", [D, T], bf16, kind="ExternalInput")
    negw1_d = nc.dram_tensor("negw1", [D, JC], f32, kind="ExternalInput")
    posw1_d = nc.dram_tensor("posw1", [D, JC], f32, kind="ExternalInput")
    beta_d = nc.dram_tensor("beta", [JC], f32, kind="ExternalInput")
    negw2_d = nc.dram_tensor("negw2", [JC, M], f32, kind="ExternalInput")
    posw2_d = nc.dram_tensor("posw2", [JC, M], f32, kind="ExternalInput")
    bo_d = nc.dram_tensor("bo", [M], f32, kind="ExternalInput")
    out_d = nc.dram_tensor("out", [MS, T], f32, kind="ExternalOutput")

    with tile.TileContext(nc) as tc, ExitStack() as ctx:
        cpool = ctx.enter_context(tc.tile_pool(name="const", bufs=1))
        apool = ctx.enter_context(tc.tile_pool(name="abs", bufs=8))
        spool = ctx.enter_context(tc.tile_pool(name="spk", bufs=1))
        opool = ctx.enter_context(tc.tile_pool(name="out", bufs=1))
        ppool = ctx.enter_context(tc.tile_pool(name="psum", bufs=2, space="PSUM"))
        dpool = ctx.enter_context(tc.tile_pool(name="dram", bufs=1, space="DRAM"))

        # ---- constants / inputs to SBUF ----
        x_sb = []
        negw1_sb = []
        posw1_sb = []
        for db in range(n_dblk):
            t = cpool.tile([128, T], bf16, tag=f"x{db}")
            nc.sync.dma_start(t[:], xT_d[db * 128:(db + 1) * 128, :])
            x_sb.append(t)
            w = cpool.tile([128, JC], f32, tag=f"w1{db}")
            nc.sync.dma_start(w[:], negw1_d[db * 128:(db + 1) * 128, :])
            negw1_sb.append(w)
            w = cpool.tile([128, JC], f32, tag=f"pw1{db}")
            nc.sync.dma_start(w[:], posw1_d[db * 128:(db + 1) * 128, :])
            posw1_sb.append(w)
        negw2_sb = []
        posw2_sb = []
        beta_sb = []
        spikes = []
        beta2d = beta_d.ap().rearrange("(p o) -> p o", o=1)
        for jb in range(n_jblk):
            w = cpool.tile([128, M], f32, tag=f"w2{jb}")
            nc.sync.dma_start(w[:], negw2_d[jb * 128:(jb + 1) * 128, :])
            negw2_sb.append(w)
            w = cpool.tile([128, M], f32, tag=f"pw2{jb}")
            nc.sync.dma_start(w[:], posw2_d[jb * 128:(jb + 1) * 128, :])
            posw2_sb.append(w)
            b = cpool.tile([128, 1], f32, tag=f"beta{jb}")
            nc.sync.dma_start(b[:], beta2d[jb * 128:(jb + 1) * 128, :])
            beta_sb.append(b)
            spikes.append(spool.tile([128, T], bf16, tag=f"spk{jb}", name=f"spk{jb}"))
        bo2d = bo_d.ap().rearrange("(p o) -> p o", o=1)
        bo_sb = []
        for mb in range(n_mblk):
            b = cpool.tile([128, 1], f32, tag=f"bo{mb}")
            nc.sync.dma_start(b[:], bo2d[mb * 128:(mb + 1) * 128, :])
            bo_sb.append(b)
        # single SBUF tile for all L2 partials (one DMA to the collective
        # bounce buffer -> the collective waits on a single DMA queue)
        partial_big = opool.tile([128, n_mblk * T], f32, tag="par", name="par")

        # Shifted ones-column windows: window(j, s) = matrix with a single
        # column of value s at column index j, so matmul(psum, window, rhs)
        # adds s * colsum(rhs) into PSUM row j (and zero elsewhere).
        # Separate even/odd-j tensors keep window starts 4B-aligned.
        G = cpool.tile([128, 256], bf16, tag="G")
        H = cpool.tile([128, 256], bf16, tag="H")
        Gn = cpool.tile([128, 256], bf16, tag="Gn")
        Hn = cpool.tile([128, 256], bf16, tag="Hn")
        for t_, v in ((G, 1.0), (H, 1.0), (Gn, -1.0), (Hn, -1.0)):
            nc.vector.memset(t_[:], 0.0)
        nc.vector.memset(G[:, 128:129], 1.0)
        nc.vector.memset(H[:, 127:128], 1.0)
        nc.vector.memset(Gn[:, 128:129], -1.0)
        nc.vector.memset(Hn[:, 127:128], -1.0)

        def window(j, sign=1):
            if j % 2 == 0:
                return (G if sign > 0 else Gn)[:, 128 - j:256 - j]
            return (H if sign > 0 else Hn)[:, 127 - j:255 - j]

        unit = 0

        def produce(src, neg_col, pos_col, dst_override=None):
            """Emit elementwise work for one (reduce-block, out-idx) unit.

            Returns [(tile, sign), ...] to stream through the PE.
            ACT unit: one fused Abs(src - w).  DVE unit: max(src, w) and
            min(src, w); PE forms the difference via the signed windows.
            """
            nonlocal unit
            is_dve = unit % dve_mod in dve_rem
            unit += 1
            if is_dve:
                mx = apool.tile([128, T], bf16, tag="abs", name="mx")
                nc.vector.tensor_scalar(mx[:], src[:], pos_col, None,
                                        op0=mybir.AluOpType.max)
                mn = apool.tile([128, T], bf16, tag="abs", name="mn")
                nc.vector.tensor_scalar(mn[:], src[:], pos_col, None,
                                        op0=mybir.AluOpType.min)
                return [(mx, 1), (mn, -1)]
            dst = dst_override
            if dst is None:
                dst = apool.tile([128, T], bf16, tag="abs", name="ab")
            nc.scalar.activation(dst[:], src[:], Act.Abs,
                                 bias=neg_col, scale=1.0)
            return [(dst, 1)]

        # ---- layer 1: zone/spikes for local j ----
        psum_l1_last = None
        for jb in range(n_jblk):
            psum = ppool.tile([128, T], f32, tag="ps")
            psum_l1_last = psum
            for jj in range(128):
                j = jb * 128 + jj
                for db in range(n_dblk):
                    tiles = produce(x_sb[db], negw1_sb[db][:, j:j + 1],
                                    posw1_sb[db][:, j:j + 1])
                    first = jj == 0 and db == 0
                    last = jj == 127 and db == n_dblk - 1
                    for ti, (a, sign) in enumerate(tiles):
                        for c in range(n_ch):
                            nc.tensor.matmul(
                                psum[:, c * CH:(c + 1) * CH],
                                window(jj, sign),
                                a[:, c * CH:(c + 1) * CH],
                                start=(first and ti == 0),
                                stop=(last and ti == len(tiles) - 1),
                            )
            # spikes = sigmoid(-B/D * psum + B*(b_in - v_th))
            nc.scalar.activation(spikes[jb][:], psum[:], Act.Sigmoid,
                                 bias=beta_sb[jb][:, 0:1],
                                 scale=-SURR_BETA / D)

        # ---- layer 2: partial sums over local j for all m ----
        # The first L2 unit on each engine gets a dedicated (non-recycled)
        # tile: a pooled slot would add PE+DVE release waits on top of the
        # ACT (spikes) + DMA (negw2) deps and blow the ISA sync-wait budget.
        l2first = [cpool.tile([128, T], bf16, tag=f"l2f{i}", name=f"l2f{i}")
                   for i in range(2)]
        for mb in range(n_mblk):
            psum = ppool.tile([128, T], f32, tag="ps")
            for mm in range(128):
                m = mb * 128 + mm
                for jb in range(n_jblk):
                    ovr = l2first[jb] if (mb == 0 and mm == 0 and jb < 2) else None
                    tiles = produce(spikes[jb], negw2_sb[jb][:, m:m + 1],
                                    posw2_sb[jb][:, m:m + 1], dst_override=ovr)
                    first = mm == 0 and jb == 0
                    last = mm == 127 and jb == n_jblk - 1
                    for ti, (a, sign) in enumerate(tiles):
                        for c in range(n_ch):
                            nc.tensor.matmul(
                                psum[:, c * CH:(c + 1) * CH],
                                window(mm, sign),
                                a[:, c * CH:(c + 1) * CH],
                                start=(first and ti == 0),
                                stop=(last and ti == len(tiles) - 1),
                            )
            # partial = b_out[m]/n_cores - psum/N  (summed across cores by RS)
            nc.scalar.activation(partial_big[:, mb * T:(mb + 1) * T], psum[:],
                                 Act.Identity,
                                 bias=bo_sb[mb][:, 0:1], scale=-1.0 / N)

        # ---- ReduceScatter over cores -> local m-shard ----
        bounce_in = dpool.tile([M, T], f32, tag="cin")
        bounce_out = dpool.tile([MS, T], f32, tag="cout")
        nc.sync.dma_start(
            bounce_in.rearrange("(mb p) t -> p mb t", p=128),
            partial_big.rearrange("p (mb t) -> p mb t", t=T))
        nc.gpsimd.collective_compute(
            "ReduceScatter",
            mybir.AluOpType.add,
            replica_groups=[list(range(n_cores))],
            ins=[bounce_in.opt()],
            outs=[bounce_out.opt()],
        )
        nc.sync.dma_start(out_d[:, :], bounce_out[:])

    nc.compile()
    return nc


def prep_inputs(x, W_in, b_in, W_out, b_out, v_th, n_cores=8):
    """Host-side prep: transposes, negation, folding. Returns per-core input maps."""
    import ml_dtypes

    T = x.shape[0] * x.shape[1]
    D = x.shape[2]
    N = W_in.shape[0]
    M = W_out.shape[0]
    JC = N // n_cores
    xT = np.ascontiguousarray(x.reshape(T, D).T).astype(ml_dtypes.bfloat16)
    negw1 = np.ascontiguousarray(-W_in.T.astype(np.float32))      # [D, N]
    beta = (SURR_BETA * (b_in - v_th)).astype(np.float32)         # [N]
    negw2 = np.ascontiguousarray(-W_out.T.astype(np.float32))     # [N, M]
    bo = (b_out / n_cores).astype(np.float32)                     # [M]
    in_maps = []
    for c in range(n_cores):
        sl = slice(c * JC, (c + 1) * JC)
        in_maps.append({
            "xT": xT,
            "negw1": np.ascontiguousarray(negw1[:, sl]),
            "posw1": np.ascontiguousarray(-negw1[:, sl]),
            "beta": np.ascontiguousarray(beta[sl]),
            "negw2": np.ascontiguousarray(negw2[sl, :]),
            "posw2": np.ascontiguousarray(-negw2[sl, :]),
            "bo": bo,
        })
    return in_maps


_NC_CACHE = {}


def _get_nc():
    if "nc" not in _NC_CACHE:
        _NC_CACHE["nc"] = build_kernel()
    return _NC_CACHE["nc"]


def run_on_hw(inputs, trace=False):
    """Run on the 8 NeuronCores; returns (full_output, BassKernelResults)."""
    from concourse.bass_utils import run_bass_kernel_spmd

    n_cores = 8
    nc = _get_nc()
    in_maps = prep_inputs(**inputs, n_cores=n_cores)
    res = run_bass_kernel_spmd(nc, in_maps, core_ids=list(range(n_cores)),
                               trace=trace)
    B, S, D_model = inputs["x"].shape
    T = B * S
    M = inputs["W_out"].shape[0]
    MS = M // n_cores
    full = np.empty((M, T), np.float32)
    for c in range(n_cores):
        full[c * MS:(c + 1) * MS, :] = res.results[c]["out"]
    out = np.ascontiguousarray(full.T).reshape(B, S, D_model).astype(np.float32)
    return out, res


def kernel(x, W_in, b_in, W_out, b_out, v_th):
    out, _ = run_on_hw(dict(x=x, W_in=W_in, b_in=b_in, W_out=W_out,
                            b_out=b_out, v_th=v_th))
    return out
